# revision 1
# baseline (speedup 1.0000x reference)
"""Trainium2 Bass kernel for the LoE tiled-MLP (NeRF-style coordinate net).

Sharding: data-parallel over the pixel axis. N=262144 rows are split
contiguously across 8 cores (32768 rows each). Because the per-layer
expert tiles are contiguous row blocks, each core only ever needs a
contiguous slice of every weight tensor -> zero cross-core traffic.

On-device layout: activations are feature-major [d, n] so every layer is
psum[o, n] += w[d_blk, o_blk].T @ x[d_blk, n] with w slices as the
stationary operand. Positional encoding is done on device:
  t = c * 2^(k-1) (+0.25 for cos rows)  -- one small matmul
  r = t - round(t)                      -- magic-constant round on DVE
  sin(2*pi*r)                           -- ACT engine (valid range +-pi)
LeakyReLU(0.2) is two ops (one PSUM operand max per instruction):
  r = relu(0.8*ps) on ACT, then x = 0.2*ps + r on DVE.
Chunks are emitted pairwise, layer-interleaved, so the in-order PE queue
always has an independent matmul behind each LeakyReLU-chain wait.
"""

import os
import sys

import numpy as np

sys.path.insert(0, "/opt/trn_rl_repo")

import concourse.bass as bass
import concourse.bacc as bacc
import concourse.mybir as mybir
import concourse.tile as tile
from concourse.alu_op_type import AluOpType
from concourse.bass_utils import run_bass_kernel_spmd

F32 = mybir.dt.float32
F32R = mybir.dt.float32r
ACT_SIN = mybir.ActivationFunctionType.Sin

N = 262144
NCORES = 8
ROWS = N // NCORES          # 32768 rows per core
CH = 512                    # pixels per chunk (psum free-dim, fp32 max)
K = 13                      # frequencies
H = 256
PE_SC = 2 * 2 * K + 2       # 52 sin/cos + 2 linearized coord rows
COORD_S = float(2.0 ** -11)  # tiny freq: sin(2*pi*s*c) ~ 2*pi*s*c, rel err 1.6e-6
MAGIC = float(1.5 * 2 ** 23)
TWO_PI = float(2.0 * np.pi)

# local (per-core) expert-tile row extents for layers 1..4
TILE_ROWS = {1: 65536, 2: 16384, 3: 4096, 4: 1024}

TRACE = False
LAST = {}


def _build(rows, f32r=True, stage_cols=2048, lrelu_eng=("a", "a", "a", "a", "a")):
    """Build the SPMD single-core Bass program for `rows` pixels."""
    nchunks = rows // CH
    stage_cols = min(stage_cols, rows)
    cpg = stage_cols // CH                       # chunks per DMA stage
    ntile = {l: max(rows // TILE_ROWS[l], 1) for l in (1, 2, 3, 4)}
    # chunk j -> local tile index for layer l
    tidx = {l: [min(j * CH // TILE_ROWS[l], ntile[l] - 1) for j in range(nchunks)]
            for l in (1, 2, 3, 4)}

    MDT = F32R if f32r else F32
    nc = bacc.Bacc()
    d_coords = nc.dram_tensor("coordsT3", [3, rows], F32, kind="ExternalInput")
    d_smat = nc.dram_tensor("smat", [3, PE_SC], F32, kind="ExternalInput")
    d_w0s = nc.dram_tensor("w0s", [PE_SC, H], MDT, kind="ExternalInput")
    d_wmid = {l: nc.dram_tensor(f"w{l}", [ntile[l], H, H], MDT, kind="ExternalInput")
              for l in (1, 2, 3, 4)}
    d_wl = nc.dram_tensor("wlT", [H, 3], MDT, kind="ExternalInput")
    d_out = nc.dram_tensor("out", [3, rows], F32, kind="ExternalOutput")

    def mdt(ap):
        return ap

    def lrelu(mode, xt, ps, rt):
        """xt(sbuf) = LeakyReLU_0.2(ps).  rt: scratch sbuf tile.

        Only ACT and DVE can read PSUM, and at most one tensor operand of a
        DVE op may live in PSUM, hence the two-pass forms.
        """
        if mode == "a":      # ACT relu + DVE combine
            nc.scalar.activation(rt[:], ps[:], mybir.ActivationFunctionType.Relu,
                                 scale=0.8)
            nc.vector.scalar_tensor_tensor(xt[:], ps[:], 0.2, rt[:],
                                           AluOpType.mult, AluOpType.add)
        elif mode == "v":    # DVE relu + DVE combine
            nc.vector.tensor_scalar(rt[:], ps[:], 0.0, 0.8,
                                    AluOpType.max, AluOpType.mult)
            nc.vector.scalar_tensor_tensor(xt[:], ps[:], 0.2, rt[:],
                                           AluOpType.mult, AluOpType.add)
        elif mode == "hwl":  # single ACT op, HW Lrelu table (alpha slope)
            nc.scalar.activation(xt[:], ps[:], mybir.ActivationFunctionType.Lrelu,
                                 alpha=0.2)
        elif mode == "hwp":  # single ACT op, HW Prelu table (alpha slope)
            nc.scalar.activation(xt[:], ps[:], mybir.ActivationFunctionType.Prelu,
                                 alpha=0.2)
        else:
            raise ValueError(mode)

    with tile.TileContext(nc) as tc:
        with (
            tc.tile_pool(name="wp", bufs=1) as wp,
            tc.tile_pool(name="io", bufs=2) as iop,
            tc.tile_pool(name="ac", bufs=2) as acp,
            tc.tile_pool(name="psa", bufs=2, space="PSUM") as ppa,
            tc.tile_pool(name="psb", bufs=3, space="PSUM") as ppb,
        ):
            # ---- resident weights (DMA once, first-use order) ----
            smat_sb = wp.tile([3, PE_SC], F32, tag="smat")
            nc.sync.dma_start(out=smat_sb[:], in_=d_smat[:])
            w0s_sb = wp.tile([PE_SC, H], MDT, tag="w0s")
            nc.sync.dma_start(out=w0s_sb[:], in_=d_w0s[:])
            wl_sb = []
            for kb in range(2):
                t = wp.tile([128, 3], MDT, tag=f"wl{kb}")
                nc.sync.dma_start(out=t[:], in_=d_wl[kb * 128:(kb + 1) * 128, :])
                wl_sb.append(t)

            wmid_sb = {l: [[None, None] for _ in range(ntile[l])] for l in (1, 2, 3, 4)}
            order = []
            for l in (1, 2, 3, 4):
                for t in range(ntile[l]):
                    first = min(j for j in range(nchunks) if tidx[l][j] == t)
                    order.append((first, l, t))
            order.sort()
            for _, l, t in order:
                for kb in range(2):
                    w = wp.tile([128, H], MDT, tag=f"w{l}_{t}_{kb}")
                    nc.sync.dma_start(
                        out=w[:], in_=d_wmid[l][t, kb * 128:(kb + 1) * 128, :])
                    wmid_sb[l][t][kb] = w

            # ---- main chunk loop: pairs of chunks, layer-interleaved ----
            # PE is an in-order queue: emitting chunk j+1's matmuls right
            # after chunk j's same-layer matmuls means every PE wait (on the
            # LeakyReLU chain) has independent work queued behind it.
            cr = None
            ot = None
            st = {}
            for jj in range(0, nchunks, 2):
                pair = [j for j in (jj, jj + 1) if j < nchunks]
                for j in pair:
                    g, o = divmod(j, cpg)
                    if o == 0:
                        cr = iop.tile([3, stage_cols], F32, tag="cr")
                        nc.sync.dma_start(
                            out=cr[:],
                            in_=d_coords[:, g * stage_cols:(g + 1) * stage_cols])
                        ot = iop.tile([3, stage_cols], F32, tag="ot")
                    rc = cr[:, o * CH:(o + 1) * CH]      # [3, 512] coords+ones
                    tps = ppa.tile([PE_SC, CH], F32, tag="ang")
                    nc.tensor.matmul(tps[:], smat_sb[:], rc, start=True, stop=True)
                    st[j] = {"rc": rc, "tps": tps, "ot": ot, "g": g, "o": o}
                for j in pair:
                    s = st[j]
                    rnd = acp.tile([PE_SC, CH], F32, tag="rnd")
                    nc.vector.tensor_scalar(rnd[:], s["tps"][:], MAGIC, MAGIC,
                                            AluOpType.add, AluOpType.subtract)
                    frac = acp.tile([PE_SC, CH], F32, tag="frac")
                    nc.vector.tensor_tensor(frac[:], s["tps"][:], rnd[:],
                                            AluOpType.subtract)
                    sc = acp.tile([PE_SC, CH], MDT, tag="sc")
                    nc.scalar.activation(sc[:], frac[:], ACT_SIN, scale=TWO_PI)
                    s["sc"] = sc
                for j in pair:
                    s = st[j]
                    ps = ppb.tile([128, 2 * CH], F32, tag="lps")
                    for ob in range(2):
                        nc.tensor.matmul(ps[:, ob * CH:(ob + 1) * CH],
                                         w0s_sb[:, ob * 128:(ob + 1) * 128],
                                         s["sc"][:], start=True, stop=True)
                    x = acp.tile([128, 2 * CH], MDT, tag="x0")
                    rt = acp.tile([128, 2 * CH], F32, tag="rt")
                    lrelu(lrelu_eng[0], x, ps, rt)
                    s["x"] = x
                for l in (1, 2, 3, 4):
                    for j in pair:
                        s = st[j]
                        wt = wmid_sb[l][tidx[l][j]]
                        ps = ppb.tile([128, 2 * CH], F32, tag="lps")
                        for ob in range(2):
                            osl = slice(ob * CH, (ob + 1) * CH)
                            wsl = slice(ob * 128, (ob + 1) * 128)
                            for kb in range(2):
                                nc.tensor.matmul(
                                    ps[:, osl], wt[kb][:, wsl],
                                    s["x"][:, kb * CH:(kb + 1) * CH],
                                    start=(kb == 0), stop=(kb == 1))
                        xn = acp.tile([128, 2 * CH], MDT, tag=f"x{l}")
                        rt = acp.tile([128, 2 * CH], F32, tag="rt")
                        lrelu(lrelu_eng[l], xn, ps, rt)
                        s["x"] = xn
                for j in pair:
                    s = st[j]
                    po = ppb.tile([3, CH], F32, tag="lps")
                    for kb in range(2):
                        nc.tensor.matmul(po[:], wl_sb[kb][:],
                                         s["x"][:, kb * CH:(kb + 1) * CH],
                                         start=(kb == 0), stop=(kb == 1))
                    nc.scalar.copy(s["ot"][:, s["o"] * CH:(s["o"] + 1) * CH], po[:])
                    if s["o"] == cpg - 1:
                        nc.sync.dma_start(
                            out=d_out[:, s["g"] * stage_cols:(s["g"] + 1) * stage_cols],
                            in_=s["ot"][:])
                    del st[j]
    nc.finalize()
    return nc


def _host_prep(coords, w0, w1, w2, w3, w4, w_last, rows):
    """Split full inputs into per-core in_maps."""
    coords = np.asarray(coords, np.float32)
    smat = np.zeros((3, PE_SC), np.float32)
    for p in range(PE_SC - 2):
        k, f, s = p >> 2, (p >> 1) & 1, p & 1
        smat[f, p] = float(2.0 ** (k - 1))
        smat[2, p] = 0.25 if s else 0.0
    smat[0, PE_SC - 2] = COORD_S
    smat[1, PE_SC - 1] = COORD_S
    w0 = np.asarray(w0, np.float32)[0]              # [54, 256]
    w0s = np.empty((PE_SC, H), np.float32)
    w0s[:PE_SC - 2] = w0[2:]
    w0s[PE_SC - 2:] = w0[0:2] / np.float32(2.0 * np.pi * COORD_S)
    wlT = np.ascontiguousarray(np.asarray(w_last, np.float32).T)  # [256, 3]
    wmid_full = {1: np.asarray(w1, np.float32), 2: np.asarray(w2, np.float32),
                 3: np.asarray(w3, np.float32), 4: np.asarray(w4, np.float32)}
    ntile = {l: max(rows // TILE_ROWS[l], 1) for l in (1, 2, 3, 4)}
    in_maps = []
    for c in range(NCORES):
        sl = coords[c * rows:(c + 1) * rows]
        ct3 = np.empty((3, rows), np.float32)
        ct3[0:2] = sl.T
        ct3[2] = 1.0
        m = {"coordsT3": ct3, "smat": smat, "w0s": w0s, "wlT": wlT}
        for l in (1, 2, 3, 4):
            w = wmid_full[l]
            t0 = c * rows // (N // w.shape[0]) if w.shape[0] * rows >= N else 0
            t0 = (c * rows) // (N // w.shape[0])
            m[f"w{l}"] = np.ascontiguousarray(w[t0:t0 + ntile[l]])
        in_maps.append(m)
    return in_maps


_BUILT = {}


def kernel(coords, w0, b0, w1, b1, w2, b2, w3, b3, w4, b4, w_last, b_last,
           f32r=True, lrelu_eng=("a", "a", "a", "a", "a")):
    key = (ROWS, bool(f32r), tuple(lrelu_eng))
    if key not in _BUILT:
        _BUILT[key] = _build(ROWS, f32r=f32r, lrelu_eng=lrelu_eng)
    nc = _BUILT[key]
    in_maps = _host_prep(coords, w0, w1, w2, w3, w4, w_last, ROWS)
    res = run_bass_kernel_spmd(nc, in_maps, list(range(NCORES)), trace=TRACE)
    LAST["res"] = res
    out = np.empty((N, 3), np.float32)
    for c in range(NCORES):
        out[c * ROWS:(c + 1) * ROWS, :] = res.results[c]["out"].T
    return out



# revision 2
# speedup vs baseline: 2.0191x; 2.0191x over previous
"""Trainium2 Bass kernel v3 for the LoE tiled-MLP.

vs v2:
- Positional encoding via fp16 hi/lo coordinate split: two 1-cycle/row fp16
  matmuls accumulating in PSUM replace the 4-cycle/row fp32 matmul pair
  (angle error ~8e-4 in sin units). Posenc PSUM/elementwise are per chunk
  PAIR ([54,1024] ops) to halve op-count overhead.
- Configurable compute dtype MDT for weights/activations: fp16 / bf16 /
  float32r (ACT op throughput appears dtype-of-output sensitive).
- Per-layer LeakyReLU modes: act (1 ACT Prelu op), split (ob0 ACT Prelu,
  ob1 DVE copy + DVE max-combine), dvefull (both obs on DVE), adve.
"""

import numpy as np
import sys

sys.path.insert(0, "/opt/trn_rl_repo")

import concourse.bass as bass
import concourse.bacc as bacc
import concourse.mybir as mybir
import concourse.tile as tile
from concourse.alu_op_type import AluOpType
from concourse.bass_utils import run_bass_kernel_spmd

F32 = mybir.dt.float32
F16 = mybir.dt.float16
BF16 = mybir.dt.bfloat16
F32R = mybir.dt.float32r
ACT_SIN = mybir.ActivationFunctionType.Sin
PRELU = mybir.ActivationFunctionType.Prelu
RELU = mybir.ActivationFunctionType.Relu

N = 262144
NCORES = 8
ROWS = N // NCORES
CH = 512
G = 4
K = 13
H = 256
PE_SC = 2 * 2 * K + 2
COORD_S = float(2.0 ** -11)
MAGIC = float(1.5 * 2 ** 23)
TWO_PI = float(2.0 * np.pi)

TILE_ROWS = {1: 65536, 2: 16384, 3: 4096, 4: 1024}

TRACE = False
LAST = {}

_DT = {"f16": F16, "bf16": BF16, "f32r": F32R}


def _build3(rows, modes, mdt="f16", out_eng="ADAD", psum=(1, 3), gsz=G):
    nch = rows // CH
    ngr = nch // gsz
    npairs = nch // 2
    ntile = {l: max(rows // TILE_ROWS[l], 1) for l in (1, 2, 3, 4)}
    tidx = {l: [min(c * CH // TILE_ROWS[l], ntile[l] - 1) for c in range(nch)]
            for l in (1, 2, 3, 4)}
    MDT = _DT[mdt]

    nc = bacc.Bacc()
    d_c6e = nc.dram_tensor("c6e", [6, rows // 2], F16, kind="ExternalInput")
    d_c6o = nc.dram_tensor("c6o", [6, rows // 2], F16, kind="ExternalInput")
    d_smat = nc.dram_tensor("smat", [6, PE_SC], F16, kind="ExternalInput")
    d_w0s = nc.dram_tensor("w0s", [PE_SC, H], MDT, kind="ExternalInput")
    d_wmid = {l: nc.dram_tensor(f"w{l}", [ntile[l], H, H], MDT, kind="ExternalInput")
              for l in (1, 2, 3, 4)}
    d_wl = nc.dram_tensor("wlT", [H, 3], F16, kind="ExternalInput")
    d_out = nc.dram_tensor("out", [3, rows], F32, kind="ExternalOutput")

    with tile.TileContext(nc) as tc:
        with (
            tc.tile_pool(name="wp", bufs=1) as wp,
            tc.tile_pool(name="crp", bufs=gsz) as crp,
            tc.tile_pool(name="scp", bufs=gsz - 1) as scp,
            tc.tile_pool(name="rfp", bufs=gsz) as rfp,
            tc.tile_pool(name="xp", bufs=2 * gsz + 2) as xp,
            tc.tile_pool(name="x4p", bufs=gsz + 3) as x4p,
            tc.tile_pool(name="cpp", bufs=gsz + 2) as cpp,
            tc.tile_pool(name="otp", bufs=2) as otp,
            tc.tile_pool(name="posp", bufs=psum[0], space="PSUM") as posp,
            tc.tile_pool(name="midp", bufs=psum[1], space="PSUM") as midp,
        ):
            smat_sb = wp.tile([70, PE_SC], F16, tag="smat")
            nc.sync.dma_start(out=smat_sb[0:6, :], in_=d_smat[:])
            nc.sync.dma_start(out=smat_sb[64:70, :], in_=d_smat[:])
            w0s_sb = wp.tile([PE_SC, H], MDT, tag="w0s")
            nc.sync.dma_start(out=w0s_sb[:], in_=d_w0s[:])

            wmid_sb = {l: [None] * ntile[l] for l in (1, 2, 3, 4)}
            wl_sb = None

            def dma_mid(l, t):
                pair = []
                for kb in range(2):
                    w = wp.tile([128, H], MDT, tag=f"w{l}_{t}_{kb}")
                    nc.sync.dma_start(
                        out=w[:], in_=d_wmid[l][t, kb * 128:(kb + 1) * 128, :])
                    pair.append(w)
                wmid_sb[l][t] = pair

            def dma_group_weights(g):
                """Stage the expert tiles first used by group g."""
                for c in range(g * gsz, min((g + 1) * gsz, nch)):
                    for l in (1, 2, 3, 4):
                        t = tidx[l][c]
                        if wmid_sb[l][t] is None:
                            dma_mid(l, t)

            # prologue: only what groups 0-1 touch, then wl; the rest is
            # staged just-in-time inside the main loop (2 groups ahead) so
            # the first posenc/coords DMAs aren't queued behind ~5 MB.
            dma_group_weights(0)
            wl_sb = []
            for kb in range(2):
                t = wp.tile([128, 3], F16, tag=f"wl{kb}")
                nc.sync.dma_start(out=t[:], in_=d_wl[kb * 128:(kb + 1) * 128, :])
                wl_sb.append(t)
            dma_group_weights(1)

            st_sc = {}
            st_x = {}
            x4 = {}

            cr_cache = {}

            def emit_pos_elt(p):
                """Posenc (fp16 hi/lo MMs, row-packed pair) + round + sin."""
                if 2 * p >= nch:
                    return
                blk = p // 2
                if blk not in cr_cache:
                    # 2 pairs of coords, 2 DMAs: rows 0-5 = even chunks
                    # (x_hi,y_hi,1,x_lo,y_lo,0), rows 64-69 = odd chunks
                    cr = crp.tile([70, 2 * CH], F16, tag="cr")
                    bsl = slice(blk * 2 * CH, (blk + 1) * 2 * CH)
                    nc.sync.dma_start(out=cr[0:6, :], in_=d_c6e[:, bsl])
                    nc.sync.dma_start(out=cr[64:70, :], in_=d_c6o[:, bsl])
                    cr_cache[blk] = cr
                cr = cr_cache[blk]
                q = (p % 2) * CH
                tps = posp.tile([PE_SC, 2 * CH], F32, tag="tps")
                for h in range(2):
                    hsl = slice(h * CH, (h + 1) * CH)
                    psl = slice(64 * h, 64 * h + 6)
                    nc.tensor.matmul(tps[:, hsl], smat_sb[psl, :],
                                     cr[psl, q:q + CH], start=True, stop=True)
                rnd = rfp.tile([PE_SC, 2 * CH], F32, tag="rf")
                nc.vector.tensor_scalar(rnd[:], tps[:], MAGIC, MAGIC,
                                        AluOpType.add, AluOpType.subtract)
                frac = rfp.tile([PE_SC, 2 * CH], F32, tag="rf")
                nc.vector.tensor_tensor(frac[:], tps[:], rnd[:], AluOpType.subtract)
                sc = scp.tile([PE_SC, 2 * CH], MDT, tag="sc")
                nc.scalar.activation(sc[:], frac[:], ACT_SIN, scale=TWO_PI)
                st_sc[2 * p] = (sc, 0)
                st_sc[2 * p + 1] = (sc, 1)

            def emit_layer(l, c):
                pool = x4p if l == 4 else xp
                xdt = F16 if l == 4 else MDT
                x = pool.tile([128, 2 * CH], xdt, tag="x4" if l == 4 else "x")
                ps = midp.tile([128, 2 * CH], F32, tag="ps")
                m = modes[l]
                if m.startswith("mix"):
                    m = "split" if (c % gsz) < int(m[3:]) else "act"
                for ob in range(2):
                    osl = slice(ob * CH, (ob + 1) * CH)
                    wsl = slice(ob * 128, (ob + 1) * 128)
                    if l == 0:
                        sc, h = st_sc[c]
                        nc.tensor.matmul(ps[:, osl], w0s_sb[:, wsl],
                                         sc[:, h * CH:(h + 1) * CH],
                                         start=True, stop=True)
                    else:
                        wt = wmid_sb[l][tidx[l][c]]
                        x_in = st_x[c]
                        for kb in range(2):
                            nc.tensor.matmul(
                                ps[:, osl], wt[kb][:, wsl],
                                x_in[:, kb * CH:(kb + 1) * CH],
                                start=(kb == 0), stop=(kb == 1))
                    if m in ("split", "splitp"):
                        if ob == 0:
                            nc.scalar.activation(x[:, osl], ps[:, osl], PRELU,
                                                 alpha=0.2)
                        elif m == "split":
                            cc = cpp.tile([128, CH], F16 if l == 4 else MDT, tag="cc")
                            nc.vector.tensor_scalar(cc[:], ps[:, osl], 1.0, None,
                                                    AluOpType.mult)
                            nc.vector.scalar_tensor_tensor(
                                x[:, osl], cc[:], 0.2, cc[:],
                                AluOpType.mult, AluOpType.max)
                        else:
                            # splitp: DVE 0.2*ps copy; GPSIMD (fp32) relu+add
                            cc = cpp.tile([128, CH], F32, tag="ccp")
                            nc.vector.tensor_scalar(cc[:], ps[:, osl], 0.2, None,
                                                    AluOpType.mult)
                            tt = cpp.tile([128, CH], F32, tag="ttp")
                            nc.gpsimd.tensor_scalar(tt[:], cc[:], 0.0, 4.0,
                                                    AluOpType.max, AluOpType.mult)
                            nc.gpsimd.tensor_tensor(x[:, osl], cc[:], tt[:],
                                                    AluOpType.add)
                if m == "act":
                    nc.scalar.activation(x[:], ps[:], PRELU, alpha=0.2)
                elif m == "dvefull":
                    cc = cpp.tile([128, 2 * CH], F16 if l == 4 else MDT, tag="cc2")
                    nc.vector.tensor_scalar(cc[:], ps[:], 1.0, None,
                                            AluOpType.mult)
                    nc.vector.scalar_tensor_tensor(x[:], cc[:], 0.2, cc[:],
                                                   AluOpType.mult, AluOpType.max)
                elif m == "adve":
                    rt = cpp.tile([128, 2 * CH], F32, tag="rt")
                    nc.scalar.activation(rt[:], ps[:], RELU, scale=0.8)
                    nc.vector.scalar_tensor_tensor(x[:], ps[:], 0.2, rt[:],
                                                   AluOpType.mult, AluOpType.add)
                st_x[c] = x
                if l == 4:
                    x4[c] = x

            def emit_burst(g):
                ot = otp.tile([3, gsz * CH], F32, tag="ot")
                for blk in range(gsz // 4):
                    cs = [g * gsz + blk * 4 + i for i in range(4)]
                    lps = midp.tile([128, 2 * CH], F32, tag="ps")
                    for kb in range(2):
                        for i, c in enumerate(cs):
                            nc.tensor.matmul(
                                lps[32 * i:32 * i + 3, 0:CH], wl_sb[kb][:],
                                x4[c][:, kb * CH:(kb + 1) * CH],
                                start=(kb == 0), stop=(kb == 1),
                                tile_position=(0, 32 * i))
                    for i, c in enumerate(cs):
                        osl = slice((blk * 4 + i) * CH, (blk * 4 + i + 1) * CH)
                        if out_eng[i % len(out_eng)] == "A":
                            nc.scalar.copy(ot[:, osl], lps[32 * i:32 * i + 3, 0:CH])
                        else:
                            nc.vector.tensor_scalar(ot[:, osl],
                                                    lps[32 * i:32 * i + 3, 0:CH],
                                                    1.0, None, AluOpType.mult)
                        del x4[c]
                nc.sync.dma_start(out=d_out[:, g * gsz * CH:(g + 1) * gsz * CH],
                                  in_=ot[:])

            npp = gsz // 2              # pairs per group
            for p in range(npp):
                emit_pos_elt(p)
            for g in range(ngr):
                for l in range(5):
                    for c in range(g * gsz, (g + 1) * gsz):
                        emit_layer(l, c)
                    if l == 0:
                        for p in range(npp // 2):
                            emit_pos_elt(npp * (g + 1) + p)
                    elif l == 1 and g >= 1:
                        emit_burst(g - 1)
                    elif l == 2:
                        for p in range(npp // 2, npp):
                            emit_pos_elt(npp * (g + 1) + p)
                    elif l == 3:
                        dma_group_weights(g + 2)
            emit_burst(ngr - 1)
    nc.finalize()
    return nc


def _host_prep3(coords, w0, w1, w2, w3, w4, w_last, rows, mdt="f16"):
    np_mdt = {"f16": np.float16, "bf16": np.float32, "f32r": np.float32}[mdt]

    def conv(a):
        a = np.asarray(a, np.float32)
        if mdt == "bf16":
            ai = a.view(np.uint32)
            a = ((ai + 0x8000) & 0xFFFF0000).view(np.float32)
            import ml_dtypes
            return a.astype(ml_dtypes.bfloat16)
        return a.astype(np_mdt)

    coords = np.asarray(coords, np.float32)
    smat3 = np.zeros((3, PE_SC), np.float16)
    for p in range(PE_SC - 2):
        k, f, s = p >> 2, (p >> 1) & 1, p & 1
        smat3[f, p] = np.float16(2.0 ** (k - 1))
        smat3[2, p] = np.float16(0.25 if s else 0.0)
    smat3[0, PE_SC - 2] = np.float16(COORD_S)
    smat3[1, PE_SC - 1] = np.float16(COORD_S)
    smat = np.vstack([smat3, smat3])          # [6, PE_SC]
    w0 = np.asarray(w0, np.float32)[0]
    w0s = np.empty((PE_SC, H), np.float32)
    w0s[:PE_SC - 2] = w0[2:]
    w0s[PE_SC - 2:] = w0[0:2] / np.float32(2.0 * np.pi * COORD_S)
    w0s = conv(w0s)
    wlT = conv(np.ascontiguousarray(np.asarray(w_last, np.float32).T))
    wmid_full = {1: conv(w1), 2: conv(w2), 3: conv(w3), 4: conv(w4)}
    ntile = {l: max(rows // TILE_ROWS[l], 1) for l in (1, 2, 3, 4)}
    in_maps = []
    for c in range(NCORES):
        sl = coords[c * rows:(c + 1) * rows]          # [rows, 2] fp32
        hi = sl.T.astype(np.float16)                  # [2, rows]
        lo = (sl.T - hi.astype(np.float32)).astype(np.float16)
        c6 = np.zeros((6, rows), np.float16)
        c6[0:2] = hi
        c6[2] = np.float16(1.0)
        c6[3:5] = lo
        c6r = c6.reshape(6, rows // CH, CH)
        m = {"c6e": np.ascontiguousarray(c6r[:, 0::2].reshape(6, rows // 2)),
             "c6o": np.ascontiguousarray(c6r[:, 1::2].reshape(6, rows // 2)),
             "smat": smat, "w0s": w0s, "wlT": wlT}
        for l in (1, 2, 3, 4):
            w = wmid_full[l]
            t0 = (c * rows) // (N // w.shape[0])
            m[f"w{l}"] = np.ascontiguousarray(w[t0:t0 + ntile[l]])
        in_maps.append(m)
    return in_maps


_BUILT3 = {}


def kernel(coords, w0, b0, w1, b1, w2, b2, w3, b3, w4, b4, w_last, b_last,
           modes=("act", "act", "mix3", "split", "split"), mdt="f16",
           out_eng="ADAD", psum=(1, 3), gsz=G):
    key = (ROWS, tuple(modes), mdt, out_eng, psum, gsz)
    if key not in _BUILT3:
        _BUILT3[key] = _build3(ROWS, modes=modes, mdt=mdt, out_eng=out_eng,
                               psum=psum, gsz=gsz)
    nc = _BUILT3[key]
    in_maps = _host_prep3(coords, w0, w1, w2, w3, w4, w_last, ROWS, mdt=mdt)
    res = run_bass_kernel_spmd(nc, in_maps, list(range(NCORES)), trace=TRACE)
    LAST["res"] = res
    out = np.empty((N, 3), np.float32)
    for c in range(NCORES):
        out[c * ROWS:(c + 1) * ROWS, :] = res.results[c]["out"].T
    return out


# revision 3
# speedup vs baseline: 2.0646x; 1.0225x over previous
"""Trainium2 Bass kernel v3 for the LoE tiled-MLP.

vs v2:
- Positional encoding via fp16 hi/lo coordinate split: two 1-cycle/row fp16
  matmuls accumulating in PSUM replace the 4-cycle/row fp32 matmul pair
  (angle error ~8e-4 in sin units). Posenc PSUM/elementwise are per chunk
  PAIR ([54,1024] ops) to halve op-count overhead.
- Configurable compute dtype MDT for weights/activations: fp16 / bf16 /
  float32r (ACT op throughput appears dtype-of-output sensitive).
- Per-layer LeakyReLU modes: act (1 ACT Prelu op), split (ob0 ACT Prelu,
  ob1 DVE copy + DVE max-combine), dvefull (both obs on DVE), adve.
"""

import numpy as np
import sys

sys.path.insert(0, "/opt/trn_rl_repo")

import concourse.bass as bass
import concourse.bacc as bacc
import concourse.mybir as mybir
import concourse.tile as tile
from concourse.alu_op_type import AluOpType
from concourse.bass_utils import run_bass_kernel_spmd

F32 = mybir.dt.float32
F16 = mybir.dt.float16
BF16 = mybir.dt.bfloat16
F32R = mybir.dt.float32r
ACT_SIN = mybir.ActivationFunctionType.Sin
PRELU = mybir.ActivationFunctionType.Prelu
RELU = mybir.ActivationFunctionType.Relu

N = 262144
NCORES = 8
ROWS = N // NCORES
CH = 512
G = 4
K = 13
H = 256
PE_SC = 2 * 2 * K + 2
COORD_S = float(2.0 ** -11)
MAGIC = float(1.5 * 2 ** 23)
TWO_PI = float(2.0 * np.pi)

TILE_ROWS = {1: 65536, 2: 16384, 3: 4096, 4: 1024}

TRACE = False
LAST = {}

_DT = {"f16": F16, "bf16": BF16, "f32r": F32R}


def _build3(rows, modes, mdt="f16", out_eng="ADAD", psum=(1, 3), gsz=G,
            burst_l=1):
    nch = rows // CH
    ngr = nch // gsz
    npairs = nch // 2
    ntile = {l: max(rows // TILE_ROWS[l], 1) for l in (1, 2, 3, 4)}
    tidx = {l: [min(c * CH // TILE_ROWS[l], ntile[l] - 1) for c in range(nch)]
            for l in (1, 2, 3, 4)}
    MDT = _DT[mdt]

    nc = bacc.Bacc()
    d_c6e = nc.dram_tensor("c6e", [6, rows // 2], F16, kind="ExternalInput")
    d_c6o = nc.dram_tensor("c6o", [6, rows // 2], F16, kind="ExternalInput")
    d_smat = nc.dram_tensor("smat", [6, PE_SC], F16, kind="ExternalInput")
    d_w0s = nc.dram_tensor("w0s", [PE_SC, H], MDT, kind="ExternalInput")
    d_wmid = {l: nc.dram_tensor(f"w{l}", [ntile[l], H, H], MDT, kind="ExternalInput")
              for l in (1, 2, 3, 4)}
    d_wl = nc.dram_tensor("wlT", [H, 3], F16, kind="ExternalInput")
    d_out = nc.dram_tensor("out", [3, rows], F32, kind="ExternalOutput")

    with tile.TileContext(nc) as tc:
        with (
            tc.tile_pool(name="wp", bufs=1) as wp,
            tc.tile_pool(name="crp", bufs=gsz) as crp,
            tc.tile_pool(name="scp", bufs=gsz - 1) as scp,
            tc.tile_pool(name="rfp", bufs=gsz) as rfp,
            tc.tile_pool(name="xp", bufs=2 * gsz + 2) as xp,
            tc.tile_pool(name="x4p", bufs=gsz + 3) as x4p,
            tc.tile_pool(name="cpp", bufs=gsz + 2) as cpp,
            tc.tile_pool(name="otp", bufs=2) as otp,
            tc.tile_pool(name="posp", bufs=psum[0], space="PSUM") as posp,
            tc.tile_pool(name="midp", bufs=psum[1], space="PSUM") as midp,
        ):
            smat_sb = wp.tile([70, PE_SC], F16, tag="smat")
            nc.sync.dma_start(out=smat_sb[0:6, :], in_=d_smat[:])
            nc.sync.dma_start(out=smat_sb[64:70, :], in_=d_smat[:])
            w0s_sb = wp.tile([PE_SC, H], MDT, tag="w0s")
            nc.sync.dma_start(out=w0s_sb[:], in_=d_w0s[:])

            wmid_sb = {l: [None] * ntile[l] for l in (1, 2, 3, 4)}
            wl_sb = None

            def dma_mid(l, t):
                pair = []
                for kb in range(2):
                    w = wp.tile([128, H], MDT, tag=f"w{l}_{t}_{kb}")
                    nc.sync.dma_start(
                        out=w[:], in_=d_wmid[l][t, kb * 128:(kb + 1) * 128, :])
                    pair.append(w)
                wmid_sb[l][t] = pair

            def dma_group_weights(g):
                """Stage the expert tiles first used by group g."""
                for c in range(g * gsz, min((g + 1) * gsz, nch)):
                    for l in (1, 2, 3, 4):
                        t = tidx[l][c]
                        if wmid_sb[l][t] is None:
                            dma_mid(l, t)

            cr_cache = {}

            def stage_cr(blk):
                if blk in cr_cache or blk * 4 * CH >= rows:
                    return
                # 2 pairs of coords, 2 DMAs: rows 0-5 = even chunks
                # (x_hi,y_hi,1,x_lo,y_lo,0), rows 64-69 = odd chunks
                cr = crp.tile([70, 2 * CH], F16, tag="cr")
                bsl = slice(blk * 2 * CH, (blk + 1) * 2 * CH)
                nc.sync.dma_start(out=cr[0:6, :], in_=d_c6e[:, bsl])
                nc.sync.dma_start(out=cr[64:70, :], in_=d_c6o[:, bsl])
                cr_cache[blk] = cr

            # prologue: coords for the first two blocks go FIRST so the
            # posenc pipeline starts while weights stream; then only what
            # groups 0-1 touch (rest staged just-in-time, 2 groups ahead).
            stage_cr(0)
            stage_cr(1)
            dma_group_weights(0)
            wl_sb = []
            for kb in range(2):
                t = wp.tile([128, 3], F16, tag=f"wl{kb}")
                nc.sync.dma_start(out=t[:], in_=d_wl[kb * 128:(kb + 1) * 128, :])
                wl_sb.append(t)
            dma_group_weights(1)

            st_sc = {}
            st_x = {}
            x4 = {}

            pos_pend = {}

            def emit_pos_mm(p):
                """Posenc matmuls (fp16 hi/lo, row-packed pair)."""
                if 2 * p >= nch:
                    return
                blk = p // 2
                stage_cr(blk)
                cr = cr_cache[blk]
                q = (p % 2) * CH
                tps = posp.tile([PE_SC, 2 * CH], F32, tag="tps")
                for h in range(2):
                    hsl = slice(h * CH, (h + 1) * CH)
                    psl = slice(64 * h, 64 * h + 6)
                    nc.tensor.matmul(tps[:, hsl], smat_sb[psl, :],
                                     cr[psl, q:q + CH], start=True, stop=True)
                pos_pend[p] = tps

            def emit_pos_sin(p):
                """Range-reduce + sin for a staged pair (ACT-queue friendly)."""
                if p not in pos_pend:
                    return
                tps = pos_pend.pop(p)
                rnd = rfp.tile([PE_SC, 2 * CH], F32, tag="rf")
                nc.vector.tensor_scalar(rnd[:], tps[:], MAGIC, MAGIC,
                                        AluOpType.add, AluOpType.subtract)
                frac = rfp.tile([PE_SC, 2 * CH], F32, tag="rf")
                nc.vector.tensor_tensor(frac[:], tps[:], rnd[:], AluOpType.subtract)
                sc = scp.tile([PE_SC, 2 * CH], MDT, tag="sc")
                nc.scalar.activation(sc[:], frac[:], ACT_SIN, scale=TWO_PI)
                st_sc[2 * p] = (sc, 0)
                st_sc[2 * p + 1] = (sc, 1)

            def emit_pos_elt(p):
                emit_pos_mm(p)
                emit_pos_sin(p)

            def emit_layer(l, c):
                pool = x4p if l == 4 else xp
                xdt = F16 if l == 4 else MDT
                x = pool.tile([128, 2 * CH], xdt, tag="x4" if l == 4 else "x")
                ps = midp.tile([128, 2 * CH], F32, tag="ps")
                m = modes[l]
                if m.startswith("mix"):
                    m = "split" if (c % gsz) < int(m[3:]) else "act"
                for ob in range(2):
                    osl = slice(ob * CH, (ob + 1) * CH)
                    wsl = slice(ob * 128, (ob + 1) * 128)
                    if l == 0:
                        sc, h = st_sc[c]
                        nc.tensor.matmul(ps[:, osl], w0s_sb[:, wsl],
                                         sc[:, h * CH:(h + 1) * CH],
                                         start=True, stop=True)
                    else:
                        wt = wmid_sb[l][tidx[l][c]]
                        x_in = st_x[c]
                        for kb in range(2):
                            nc.tensor.matmul(
                                ps[:, osl], wt[kb][:, wsl],
                                x_in[:, kb * CH:(kb + 1) * CH],
                                start=(kb == 0), stop=(kb == 1))
                    if m in ("split", "splitp"):
                        if ob == 0:
                            nc.scalar.activation(x[:, osl], ps[:, osl], PRELU,
                                                 alpha=0.2)
                        elif m == "split":
                            cc = cpp.tile([128, CH], F16 if l == 4 else MDT, tag="cc")
                            nc.vector.tensor_scalar(cc[:], ps[:, osl], 1.0, None,
                                                    AluOpType.mult)
                            nc.vector.scalar_tensor_tensor(
                                x[:, osl], cc[:], 0.2, cc[:],
                                AluOpType.mult, AluOpType.max)
                        else:
                            # splitp: DVE 0.2*ps copy; GPSIMD (fp32) relu+add
                            cc = cpp.tile([128, CH], F32, tag="ccp")
                            nc.vector.tensor_scalar(cc[:], ps[:, osl], 0.2, None,
                                                    AluOpType.mult)
                            tt = cpp.tile([128, CH], F32, tag="ttp")
                            nc.gpsimd.tensor_scalar(tt[:], cc[:], 0.0, 4.0,
                                                    AluOpType.max, AluOpType.mult)
                            nc.gpsimd.tensor_tensor(x[:, osl], cc[:], tt[:],
                                                    AluOpType.add)
                if m == "act":
                    nc.scalar.activation(x[:], ps[:], PRELU, alpha=0.2)
                elif m == "dvefull":
                    cc = cpp.tile([128, 2 * CH], F16 if l == 4 else MDT, tag="cc2")
                    nc.vector.tensor_scalar(cc[:], ps[:], 1.0, None,
                                            AluOpType.mult)
                    nc.vector.scalar_tensor_tensor(x[:], cc[:], 0.2, cc[:],
                                                   AluOpType.mult, AluOpType.max)
                elif m == "adve":
                    rt = cpp.tile([128, 2 * CH], F32, tag="rt")
                    nc.scalar.activation(rt[:], ps[:], RELU, scale=0.8)
                    nc.vector.scalar_tensor_tensor(x[:], ps[:], 0.2, rt[:],
                                                   AluOpType.mult, AluOpType.add)
                st_x[c] = x
                if l == 4:
                    x4[c] = x

            def emit_burst(g):
                ot = otp.tile([3, gsz * CH], F32, tag="ot")
                for blk in range(gsz // 4):
                    cs = [g * gsz + blk * 4 + i for i in range(4)]
                    lps = midp.tile([128, 2 * CH], F32, tag="ps")
                    for kb in range(2):
                        for i, c in enumerate(cs):
                            nc.tensor.matmul(
                                lps[32 * i:32 * i + 3, 0:CH], wl_sb[kb][:],
                                x4[c][:, kb * CH:(kb + 1) * CH],
                                start=(kb == 0), stop=(kb == 1),
                                tile_position=(0, 32 * i))
                    for i, c in enumerate(cs):
                        osl = slice((blk * 4 + i) * CH, (blk * 4 + i + 1) * CH)
                        if out_eng[i % len(out_eng)] == "A":
                            nc.scalar.copy(ot[:, osl], lps[32 * i:32 * i + 3, 0:CH])
                        else:
                            nc.vector.tensor_scalar(ot[:, osl],
                                                    lps[32 * i:32 * i + 3, 0:CH],
                                                    1.0, None, AluOpType.mult)
                        del x4[c]
                nc.sync.dma_start(out=d_out[:, g * gsz * CH:(g + 1) * gsz * CH],
                                  in_=ot[:])

            npp = gsz // 2              # pairs per group
            for p in range(npp):
                emit_pos_elt(p)
            for g in range(ngr):
                for l in range(5):
                    for c in range(g * gsz, (g + 1) * gsz):
                        emit_layer(l, c)
                    if l == 0:
                        for p in range(npp // 2):
                            emit_pos_mm(npp * (g + 1) + p)
                    elif l == 1:
                        if g >= 1:
                            emit_burst(g - 1)
                        for p in range(npp // 2):
                            emit_pos_sin(npp * (g + 1) + p)
                    elif l == 2:
                        for p in range(npp // 2, npp):
                            emit_pos_mm(npp * (g + 1) + p)
                    elif l == 3:
                        for p in range(npp // 2, npp):
                            emit_pos_sin(npp * (g + 1) + p)
                        dma_group_weights(g + 2)
            emit_burst(ngr - 1)
    nc.finalize()
    return nc


def _host_prep3(coords, w0, w1, w2, w3, w4, w_last, rows, mdt="f16"):
    np_mdt = {"f16": np.float16, "bf16": np.float32, "f32r": np.float32}[mdt]

    def conv(a):
        a = np.asarray(a, np.float32)
        if mdt == "bf16":
            ai = a.view(np.uint32)
            a = ((ai + 0x8000) & 0xFFFF0000).view(np.float32)
            import ml_dtypes
            return a.astype(ml_dtypes.bfloat16)
        return a.astype(np_mdt)

    coords = np.asarray(coords, np.float32)
    smat3 = np.zeros((3, PE_SC), np.float16)
    for p in range(PE_SC - 2):
        k, f, s = p >> 2, (p >> 1) & 1, p & 1
        smat3[f, p] = np.float16(2.0 ** (k - 1))
        smat3[2, p] = np.float16(0.25 if s else 0.0)
    smat3[0, PE_SC - 2] = np.float16(COORD_S)
    smat3[1, PE_SC - 1] = np.float16(COORD_S)
    smat = np.vstack([smat3, smat3])          # [6, PE_SC]
    w0 = np.asarray(w0, np.float32)[0]
    w0s = np.empty((PE_SC, H), np.float32)
    w0s[:PE_SC - 2] = w0[2:]
    w0s[PE_SC - 2:] = w0[0:2] / np.float32(2.0 * np.pi * COORD_S)
    w0s = conv(w0s)
    wlT = conv(np.ascontiguousarray(np.asarray(w_last, np.float32).T))
    wmid_full = {1: conv(w1), 2: conv(w2), 3: conv(w3), 4: conv(w4)}
    ntile = {l: max(rows // TILE_ROWS[l], 1) for l in (1, 2, 3, 4)}
    in_maps = []
    for c in range(NCORES):
        sl = coords[c * rows:(c + 1) * rows]          # [rows, 2] fp32
        hi = sl.T.astype(np.float16)                  # [2, rows]
        lo = (sl.T - hi.astype(np.float32)).astype(np.float16)
        c6 = np.zeros((6, rows), np.float16)
        c6[0:2] = hi
        c6[2] = np.float16(1.0)
        c6[3:5] = lo
        c6r = c6.reshape(6, rows // CH, CH)
        m = {"c6e": np.ascontiguousarray(c6r[:, 0::2].reshape(6, rows // 2)),
             "c6o": np.ascontiguousarray(c6r[:, 1::2].reshape(6, rows // 2)),
             "smat": smat, "w0s": w0s, "wlT": wlT}
        for l in (1, 2, 3, 4):
            w = wmid_full[l]
            t0 = (c * rows) // (N // w.shape[0])
            m[f"w{l}"] = np.ascontiguousarray(w[t0:t0 + ntile[l]])
        in_maps.append(m)
    return in_maps


_BUILT3 = {}


def kernel(coords, w0, b0, w1, b1, w2, b2, w3, b3, w4, b4, w_last, b_last,
           modes=("act", "act", "mix3", "split", "split"), mdt="f16",
           out_eng="ADAD", psum=(1, 3), gsz=G, burst_l=1):
    key = (ROWS, tuple(modes), mdt, out_eng, psum, gsz, burst_l)
    if key not in _BUILT3:
        _BUILT3[key] = _build3(ROWS, modes=modes, mdt=mdt, out_eng=out_eng,
                               psum=psum, gsz=gsz, burst_l=burst_l)
    nc = _BUILT3[key]
    in_maps = _host_prep3(coords, w0, w1, w2, w3, w4, w_last, ROWS, mdt=mdt)
    res = run_bass_kernel_spmd(nc, in_maps, list(range(NCORES)), trace=TRACE)
    LAST["res"] = res
    out = np.empty((N, 3), np.float32)
    for c in range(NCORES):
        out[c * ROWS:(c + 1) * ROWS, :] = res.results[c]["out"].T
    return out


# revision 4
# speedup vs baseline: 2.2040x; 1.0676x over previous
"""Trainium2 Bass kernel v3 for the LoE tiled-MLP.

vs v2:
- Positional encoding via fp16 hi/lo coordinate split: two 1-cycle/row fp16
  matmuls accumulating in PSUM replace the 4-cycle/row fp32 matmul pair
  (angle error ~8e-4 in sin units). Posenc PSUM/elementwise are per chunk
  PAIR ([54,1024] ops) to halve op-count overhead.
- Configurable compute dtype MDT for weights/activations: fp16 / bf16 /
  float32r (ACT op throughput appears dtype-of-output sensitive).
- Per-layer LeakyReLU modes: act (1 ACT Prelu op), split (ob0 ACT Prelu,
  ob1 DVE copy + DVE max-combine), dvefull (both obs on DVE), adve.
"""

import numpy as np
import sys

sys.path.insert(0, "/opt/trn_rl_repo")

import concourse.bass as bass
import concourse.bacc as bacc
import concourse.mybir as mybir
import concourse.tile as tile
from concourse.alu_op_type import AluOpType
from concourse.bass_utils import run_bass_kernel_spmd

F32 = mybir.dt.float32
F16 = mybir.dt.float16
BF16 = mybir.dt.bfloat16
F32R = mybir.dt.float32r
ACT_SIN = mybir.ActivationFunctionType.Sin
PRELU = mybir.ActivationFunctionType.Prelu
RELU = mybir.ActivationFunctionType.Relu

N = 262144
NCORES = 8
ROWS = N // NCORES
CH = 512
G = 4
K = 13
H = 256
PE_SC = 2 * 2 * K + 2
COORD_S = float(2.0 ** -11)
MAGIC = float(1.5 * 2 ** 23)
TWO_PI = float(2.0 * np.pi)

TILE_ROWS = {1: 65536, 2: 16384, 3: 4096, 4: 1024}

TRACE = False
LAST = {}

_DT = {"f16": F16, "bf16": BF16, "f32r": F32R}


def _build3(rows, modes, mdt="f16", out_eng="ADAD", psum=(1, 3), gsz=G,
            burst_l=1):
    nch = rows // CH
    ngr = nch // gsz
    npairs = nch // 2
    ntile = {l: max(rows // TILE_ROWS[l], 1) for l in (1, 2, 3, 4)}
    tidx = {l: [min(c * CH // TILE_ROWS[l], ntile[l] - 1) for c in range(nch)]
            for l in (1, 2, 3, 4)}
    MDT = _DT[mdt]

    nc = bacc.Bacc()
    d_c6e = nc.dram_tensor("c6e", [6, rows // 2], F16, kind="ExternalInput")
    d_c6o = nc.dram_tensor("c6o", [6, rows // 2], F16, kind="ExternalInput")
    d_smat = nc.dram_tensor("smat", [6, PE_SC], F16, kind="ExternalInput")
    d_w0s = nc.dram_tensor("w0s", [PE_SC, H], MDT, kind="ExternalInput")
    d_wmid = {l: nc.dram_tensor(f"w{l}", [ntile[l], H, H], MDT, kind="ExternalInput")
              for l in (1, 2, 3, 4)}
    d_wl = nc.dram_tensor("wlT", [H, 3], F16, kind="ExternalInput")
    d_out = nc.dram_tensor("out", [3, rows], F32, kind="ExternalOutput")

    with tile.TileContext(nc) as tc:
        with (
            tc.tile_pool(name="wp", bufs=1) as wp,
            tc.tile_pool(name="crp", bufs=gsz) as crp,
            tc.tile_pool(name="scp", bufs=gsz - 1) as scp,
            tc.tile_pool(name="rfp", bufs=gsz) as rfp,
            tc.tile_pool(name="xp", bufs=2 * gsz + 2) as xp,
            tc.tile_pool(name="x4p", bufs=gsz + 3) as x4p,
            tc.tile_pool(name="cpp", bufs=gsz + 2) as cpp,
            tc.tile_pool(name="otp", bufs=2) as otp,
            tc.tile_pool(name="posp", bufs=psum[0], space="PSUM") as posp,
            tc.tile_pool(name="midp", bufs=psum[1], space="PSUM") as midp,
        ):
            smat_sb = wp.tile([70, PE_SC], F16, tag="smat")
            nc.sync.dma_start(out=smat_sb[0:6, :], in_=d_smat[:])
            nc.sync.dma_start(out=smat_sb[64:70, :], in_=d_smat[:])
            w0s_sb = wp.tile([PE_SC, H], MDT, tag="w0s")
            nc.sync.dma_start(out=w0s_sb[:], in_=d_w0s[:])

            wmid_sb = {l: [None] * ntile[l] for l in (1, 2, 3, 4)}
            wl_sb = None

            def dma_mid(l, t):
                pair = []
                for kb in range(2):
                    w = wp.tile([128, H], MDT, tag=f"w{l}_{t}_{kb}")
                    nc.sync.dma_start(
                        out=w[:], in_=d_wmid[l][t, kb * 128:(kb + 1) * 128, :])
                    pair.append(w)
                wmid_sb[l][t] = pair

            def dma_group_weights(g):
                """Stage the expert tiles first used by group g."""
                for c in range(g * gsz, min((g + 1) * gsz, nch)):
                    for l in (1, 2, 3, 4):
                        t = tidx[l][c]
                        if wmid_sb[l][t] is None:
                            dma_mid(l, t)

            cr_cache = {}

            def stage_cr(blk):
                if blk in cr_cache or blk * 4 * CH >= rows:
                    return
                # 2 pairs of coords, 2 DMAs: rows 0-5 = even chunks
                # (x_hi,y_hi,1,x_lo,y_lo,0), rows 64-69 = odd chunks
                cr = crp.tile([70, 2 * CH], F16, tag="cr")
                bsl = slice(blk * 2 * CH, (blk + 1) * 2 * CH)
                nc.sync.dma_start(out=cr[0:6, :], in_=d_c6e[:, bsl])
                nc.sync.dma_start(out=cr[64:70, :], in_=d_c6o[:, bsl])
                cr_cache[blk] = cr

            # prologue: coords for the first two blocks go FIRST so the
            # posenc pipeline starts while weights stream; then only what
            # groups 0-1 touch (rest staged just-in-time, 2 groups ahead).
            stage_cr(0)
            stage_cr(1)
            dma_group_weights(0)
            wl_sb = []
            for kb in range(2):
                t = wp.tile([128, 3], F16, tag=f"wl{kb}")
                nc.sync.dma_start(out=t[:], in_=d_wl[kb * 128:(kb + 1) * 128, :])
                wl_sb.append(t)
            dma_group_weights(1)

            st_sc = {}
            st_x = {}
            x4 = {}

            pos_pend = {}

            def emit_pos_mm(p):
                """Posenc matmuls (fp16 hi/lo, row-packed pair)."""
                if 2 * p >= nch:
                    return
                blk = p // 2
                stage_cr(blk)
                cr = cr_cache[blk]
                q = (p % 2) * CH
                tps = posp.tile([PE_SC, 2 * CH], F32, tag="tps")
                for h in range(2):
                    hsl = slice(h * CH, (h + 1) * CH)
                    psl = slice(64 * h, 64 * h + 6)
                    nc.tensor.matmul(tps[:, hsl], smat_sb[psl, :],
                                     cr[psl, q:q + CH], start=True, stop=True)
                pos_pend[p] = tps

            def emit_pos_sin(p):
                """Range-reduce + sin for a staged pair (ACT-queue friendly)."""
                if p not in pos_pend:
                    return
                tps = pos_pend.pop(p)
                rnd = rfp.tile([PE_SC, 2 * CH], F32, tag="rf")
                nc.vector.tensor_scalar(rnd[:], tps[:], MAGIC, MAGIC,
                                        AluOpType.add, AluOpType.subtract)
                frac = rfp.tile([PE_SC, 2 * CH], F32, tag="rf")
                nc.vector.tensor_tensor(frac[:], tps[:], rnd[:], AluOpType.subtract)
                sc = scp.tile([PE_SC, 2 * CH], MDT, tag="sc")
                nc.scalar.activation(sc[:], frac[:], ACT_SIN, scale=TWO_PI)
                st_sc[2 * p] = (sc, 0)
                st_sc[2 * p + 1] = (sc, 1)

            def emit_pos_elt(p):
                emit_pos_mm(p)
                emit_pos_sin(p)

            def emit_layer(l, c):
                pool = x4p if l == 4 else xp
                xdt = F16 if l == 4 else MDT
                x = pool.tile([128, 2 * CH], xdt, tag="x4" if l == 4 else "x")
                ps = midp.tile([128, 2 * CH], F32, tag="ps")
                m = modes[l]
                if m.startswith("mix"):
                    m = "split" if (c % gsz) < int(m[3:]) else "act"
                for ob in range(2):
                    osl = slice(ob * CH, (ob + 1) * CH)
                    wsl = slice(ob * 128, (ob + 1) * 128)
                    if l == 0:
                        sc, h = st_sc[c]
                        nc.tensor.matmul(ps[:, osl], w0s_sb[:, wsl],
                                         sc[:, h * CH:(h + 1) * CH],
                                         start=True, stop=True)
                    else:
                        wt = wmid_sb[l][tidx[l][c]]
                        x_in = st_x[c]
                        for kb in range(2):
                            nc.tensor.matmul(
                                ps[:, osl], wt[kb][:, wsl],
                                x_in[:, kb * CH:(kb + 1) * CH],
                                start=(kb == 0), stop=(kb == 1))
                    if m in ("split", "splitp"):
                        if ob == 0:
                            nc.scalar.activation(x[:, osl], ps[:, osl], PRELU,
                                                 alpha=0.2)
                        elif m == "split":
                            cc = cpp.tile([128, CH], F16 if l == 4 else MDT, tag="cc")
                            nc.vector.tensor_scalar(cc[:], ps[:, osl], 1.0, None,
                                                    AluOpType.mult)
                            nc.vector.scalar_tensor_tensor(
                                x[:, osl], cc[:], 0.2, cc[:],
                                AluOpType.mult, AluOpType.max)
                        else:
                            # splitp: DVE 0.2*ps copy; GPSIMD (fp32) relu+add
                            cc = cpp.tile([128, CH], F32, tag="ccp")
                            nc.vector.tensor_scalar(cc[:], ps[:, osl], 0.2, None,
                                                    AluOpType.mult)
                            tt = cpp.tile([128, CH], F32, tag="ttp")
                            nc.gpsimd.tensor_scalar(tt[:], cc[:], 0.0, 4.0,
                                                    AluOpType.max, AluOpType.mult)
                            nc.gpsimd.tensor_tensor(x[:, osl], cc[:], tt[:],
                                                    AluOpType.add)
                if m == "act":
                    nc.scalar.activation(x[:], ps[:], PRELU, alpha=0.2)
                elif m == "dvefull":
                    cc = cpp.tile([128, 2 * CH], F16 if l == 4 else MDT, tag="cc2")
                    nc.vector.tensor_scalar(cc[:], ps[:], 1.0, None,
                                            AluOpType.mult)
                    nc.vector.scalar_tensor_tensor(x[:], cc[:], 0.2, cc[:],
                                                   AluOpType.mult, AluOpType.max)
                elif m == "adve":
                    rt = cpp.tile([128, 2 * CH], F32, tag="rt")
                    nc.scalar.activation(rt[:], ps[:], RELU, scale=0.8)
                    nc.vector.scalar_tensor_tensor(x[:], ps[:], 0.2, rt[:],
                                                   AluOpType.mult, AluOpType.add)
                st_x[c] = x
                if l == 4:
                    x4[c] = x

            def emit_burst(g):
                for blk in range(gsz // 4):
                    cs = [g * gsz + blk * 4 + i for i in range(4)]
                    lps = midp.tile([128, 2 * CH], F32, tag="ps")
                    for kb in range(2):
                        for i, c in enumerate(cs):
                            nc.tensor.matmul(
                                lps[32 * i:32 * i + 3, 0:CH], wl_sb[kb][:],
                                x4[c][:, kb * CH:(kb + 1) * CH],
                                start=(kb == 0), stop=(kb == 1),
                                tile_position=(0, 32 * i))
                    # evacuate all 4 col-groups in ONE copy (partition count
                    # is free on DVE); per-chunk DMAs pick out rows 32i..32i+2
                    ot = otp.tile([99, CH], F32, tag="ot")
                    nc.vector.tensor_scalar(ot[:], lps[0:99, 0:CH], 1.0, None,
                                            AluOpType.mult)
                    for i, c in enumerate(cs):
                        nc.sync.dma_start(out=d_out[:, c * CH:(c + 1) * CH],
                                          in_=ot[32 * i:32 * i + 3, :])
                        del x4[c]

            npp = gsz // 2              # pairs per group
            for p in range(npp):
                emit_pos_elt(p)
            for g in range(ngr):
                for l in range(5):
                    for c in range(g * gsz, (g + 1) * gsz):
                        emit_layer(l, c)
                    if l == 0:
                        for p in range(npp // 2):
                            emit_pos_mm(npp * (g + 1) + p)
                    elif l == 1:
                        if g >= 1:
                            emit_burst(g - 1)
                        for p in range(npp // 2):
                            emit_pos_sin(npp * (g + 1) + p)
                    elif l == 2:
                        for p in range(npp // 2, npp):
                            emit_pos_mm(npp * (g + 1) + p)
                    elif l == 3:
                        for p in range(npp // 2, npp):
                            emit_pos_sin(npp * (g + 1) + p)
                        dma_group_weights(g + 2)
            emit_burst(ngr - 1)
    nc.finalize()
    return nc


def _host_prep3(coords, w0, w1, w2, w3, w4, w_last, rows, mdt="f16"):
    np_mdt = {"f16": np.float16, "bf16": np.float32, "f32r": np.float32}[mdt]

    def conv(a):
        a = np.asarray(a, np.float32)
        if mdt == "bf16":
            ai = a.view(np.uint32)
            a = ((ai + 0x8000) & 0xFFFF0000).view(np.float32)
            import ml_dtypes
            return a.astype(ml_dtypes.bfloat16)
        return a.astype(np_mdt)

    coords = np.asarray(coords, np.float32)
    smat3 = np.zeros((3, PE_SC), np.float16)
    for p in range(PE_SC - 2):
        k, f, s = p >> 2, (p >> 1) & 1, p & 1
        smat3[f, p] = np.float16(2.0 ** (k - 1))
        smat3[2, p] = np.float16(0.25 if s else 0.0)
    smat3[0, PE_SC - 2] = np.float16(COORD_S)
    smat3[1, PE_SC - 1] = np.float16(COORD_S)
    smat = np.vstack([smat3, smat3])          # [6, PE_SC]
    w0 = np.asarray(w0, np.float32)[0]
    w0s = np.empty((PE_SC, H), np.float32)
    w0s[:PE_SC - 2] = w0[2:]
    w0s[PE_SC - 2:] = w0[0:2] / np.float32(2.0 * np.pi * COORD_S)
    w0s = conv(w0s)
    wlT = conv(np.ascontiguousarray(np.asarray(w_last, np.float32).T))
    wmid_full = {1: conv(w1), 2: conv(w2), 3: conv(w3), 4: conv(w4)}
    ntile = {l: max(rows // TILE_ROWS[l], 1) for l in (1, 2, 3, 4)}
    in_maps = []
    for c in range(NCORES):
        sl = coords[c * rows:(c + 1) * rows]          # [rows, 2] fp32
        hi = sl.T.astype(np.float16)                  # [2, rows]
        lo = (sl.T - hi.astype(np.float32)).astype(np.float16)
        c6 = np.zeros((6, rows), np.float16)
        c6[0:2] = hi
        c6[2] = np.float16(1.0)
        c6[3:5] = lo
        c6r = c6.reshape(6, rows // CH, CH)
        m = {"c6e": np.ascontiguousarray(c6r[:, 0::2].reshape(6, rows // 2)),
             "c6o": np.ascontiguousarray(c6r[:, 1::2].reshape(6, rows // 2)),
             "smat": smat, "w0s": w0s, "wlT": wlT}
        for l in (1, 2, 3, 4):
            w = wmid_full[l]
            t0 = (c * rows) // (N // w.shape[0])
            m[f"w{l}"] = np.ascontiguousarray(w[t0:t0 + ntile[l]])
        in_maps.append(m)
    return in_maps


_BUILT3 = {}


def kernel(coords, w0, b0, w1, b1, w2, b2, w3, b3, w4, b4, w_last, b_last,
           modes=("act", "act", "mix2", "split", "split"), mdt="f16",
           out_eng="ADAD", psum=(1, 3), gsz=G, burst_l=1):
    key = (ROWS, tuple(modes), mdt, out_eng, psum, gsz, burst_l)
    if key not in _BUILT3:
        _BUILT3[key] = _build3(ROWS, modes=modes, mdt=mdt, out_eng=out_eng,
                               psum=psum, gsz=gsz, burst_l=burst_l)
    nc = _BUILT3[key]
    in_maps = _host_prep3(coords, w0, w1, w2, w3, w4, w_last, ROWS, mdt=mdt)
    res = run_bass_kernel_spmd(nc, in_maps, list(range(NCORES)), trace=TRACE)
    LAST["res"] = res
    out = np.empty((N, 3), np.float32)
    for c in range(NCORES):
        out[c * ROWS:(c + 1) * ROWS, :] = res.results[c]["out"].T
    return out


# revision 5
# speedup vs baseline: 2.2072x; 1.0014x over previous
"""Trainium2 Bass kernel v3 for the LoE tiled-MLP.

vs v2:
- Positional encoding via fp16 hi/lo coordinate split: two 1-cycle/row fp16
  matmuls accumulating in PSUM replace the 4-cycle/row fp32 matmul pair
  (angle error ~8e-4 in sin units). Posenc PSUM/elementwise are per chunk
  PAIR ([54,1024] ops) to halve op-count overhead.
- Configurable compute dtype MDT for weights/activations: fp16 / bf16 /
  float32r (ACT op throughput appears dtype-of-output sensitive).
- Per-layer LeakyReLU modes: act (1 ACT Prelu op), split (ob0 ACT Prelu,
  ob1 DVE copy + DVE max-combine), dvefull (both obs on DVE), adve.
"""

import numpy as np
import sys

sys.path.insert(0, "/opt/trn_rl_repo")

import concourse.bass as bass
import concourse.bacc as bacc
import concourse.mybir as mybir
import concourse.tile as tile
from concourse.alu_op_type import AluOpType
from concourse.bass_utils import run_bass_kernel_spmd

F32 = mybir.dt.float32
F16 = mybir.dt.float16
BF16 = mybir.dt.bfloat16
F32R = mybir.dt.float32r
ACT_SIN = mybir.ActivationFunctionType.Sin
PRELU = mybir.ActivationFunctionType.Prelu
RELU = mybir.ActivationFunctionType.Relu

N = 262144
NCORES = 8
ROWS = N // NCORES
CH = 512
G = 4
K = 13
H = 256
PE_SC = 2 * 2 * K + 2
COORD_S = float(2.0 ** -11)
MAGIC = float(1.5 * 2 ** 23)
TWO_PI = float(2.0 * np.pi)

TILE_ROWS = {1: 65536, 2: 16384, 3: 4096, 4: 1024}

TRACE = False
LAST = {}

_DT = {"f16": F16, "bf16": BF16, "f32r": F32R}


def _build3(rows, modes, mdt="f16", out_eng="ADAD", psum=(1, 3), gsz=G,
            burst_l=1, bulk_dma="sync", bufbump=0):
    nch = rows // CH
    ngr = nch // gsz
    npairs = nch // 2
    ntile = {l: max(rows // TILE_ROWS[l], 1) for l in (1, 2, 3, 4)}
    tidx = {l: [min(c * CH // TILE_ROWS[l], ntile[l] - 1) for c in range(nch)]
            for l in (1, 2, 3, 4)}
    MDT = _DT[mdt]

    nc = bacc.Bacc()
    d_c6e = nc.dram_tensor("c6e", [6, rows // 2], F16, kind="ExternalInput")
    d_c6o = nc.dram_tensor("c6o", [6, rows // 2], F16, kind="ExternalInput")
    d_smat = nc.dram_tensor("smat", [6, PE_SC], F16, kind="ExternalInput")
    d_w0s = nc.dram_tensor("w0s", [PE_SC, H], MDT, kind="ExternalInput")
    d_wmid = {l: nc.dram_tensor(f"w{l}", [ntile[l], H, H], MDT, kind="ExternalInput")
              for l in (1, 2, 3, 4)}
    d_wl = nc.dram_tensor("wlT", [H, 3], F16, kind="ExternalInput")
    d_out = nc.dram_tensor("out", [3, rows], F32, kind="ExternalOutput")

    with tile.TileContext(nc) as tc:
        with (
            tc.tile_pool(name="wp", bufs=1) as wp,
            tc.tile_pool(name="crp", bufs=gsz) as crp,
            tc.tile_pool(name="scp", bufs=gsz - 1) as scp,
            tc.tile_pool(name="rfp", bufs=gsz) as rfp,
            tc.tile_pool(name="xp", bufs=2 * gsz + 2 + bufbump) as xp,
            tc.tile_pool(name="x4p", bufs=gsz + 3 + bufbump) as x4p,
            tc.tile_pool(name="cpp", bufs=gsz + 2 + bufbump) as cpp,
            tc.tile_pool(name="otp", bufs=2) as otp,
            tc.tile_pool(name="posp", bufs=max(psum[0], 1), space="PSUM") as posp,
            tc.tile_pool(name="midp", bufs=psum[1], space="PSUM") as midp,
        ):
            smat_sb = wp.tile([70, PE_SC], F16, tag="smat")
            nc.sync.dma_start(out=smat_sb[0:6, :], in_=d_smat[:])
            nc.sync.dma_start(out=smat_sb[64:70, :], in_=d_smat[:])
            w0s_sb = wp.tile([PE_SC, H], MDT, tag="w0s")
            nc.sync.dma_start(out=w0s_sb[:], in_=d_w0s[:])

            wmid_sb = {l: [None] * ntile[l] for l in (1, 2, 3, 4)}
            wl_sb = None

            bulk = nc.gpsimd if bulk_dma == "gpsimd" else nc.sync

            def dma_mid(l, t):
                pair = []
                for kb in range(2):
                    w = wp.tile([128, H], MDT, tag=f"w{l}_{t}_{kb}")
                    bulk.dma_start(
                        out=w[:], in_=d_wmid[l][t, kb * 128:(kb + 1) * 128, :])
                    pair.append(w)
                wmid_sb[l][t] = pair

            def dma_group_weights(g):
                """Stage the expert tiles first used by group g."""
                for c in range(g * gsz, min((g + 1) * gsz, nch)):
                    for l in (1, 2, 3, 4):
                        t = tidx[l][c]
                        if wmid_sb[l][t] is None:
                            dma_mid(l, t)

            cr_cache = {}

            def stage_cr(blk):
                if blk in cr_cache or blk * 4 * CH >= rows:
                    return
                # 2 pairs of coords, 2 DMAs: rows 0-5 = even chunks
                # (x_hi,y_hi,1,x_lo,y_lo,0), rows 64-69 = odd chunks
                cr = crp.tile([70, 2 * CH], F16, tag="cr")
                bsl = slice(blk * 2 * CH, (blk + 1) * 2 * CH)
                nc.sync.dma_start(out=cr[0:6, :], in_=d_c6e[:, bsl])
                nc.sync.dma_start(out=cr[64:70, :], in_=d_c6o[:, bsl])
                cr_cache[blk] = cr

            # prologue: coords for the first two blocks go FIRST so the
            # posenc pipeline starts while weights stream; then only what
            # groups 0-1 touch (rest staged just-in-time, 2 groups ahead).
            stage_cr(0)
            stage_cr(1)
            dma_group_weights(0)
            wl_sb = []
            for kb in range(2):
                t = wp.tile([128, 3], F16, tag=f"wl{kb}")
                nc.sync.dma_start(out=t[:], in_=d_wl[kb * 128:(kb + 1) * 128, :])
                wl_sb.append(t)
            dma_group_weights(1)

            st_sc = {}
            st_x = {}
            x4 = {}

            pos_pend = {}

            def emit_pos_mm(p):
                """Posenc matmuls (fp16 hi/lo, row-packed pair)."""
                if 2 * p >= nch:
                    return
                blk = p // 2
                stage_cr(blk)
                cr = cr_cache[blk]
                q = (p % 2) * CH
                if psum[0] == 0:
                    big = midp.tile([128, 2 * CH], F32, tag="ps")
                    tps = big[0:PE_SC, :]
                else:
                    tps = posp.tile([PE_SC, 2 * CH], F32, tag="tps")
                for h in range(2):
                    hsl = slice(h * CH, (h + 1) * CH)
                    psl = slice(64 * h, 64 * h + 6)
                    nc.tensor.matmul(tps[:, hsl], smat_sb[psl, :],
                                     cr[psl, q:q + CH], start=True, stop=True)
                pos_pend[p] = tps

            def emit_pos_sin(p):
                """Range-reduce + sin for a staged pair (ACT-queue friendly)."""
                if p not in pos_pend:
                    return
                tps = pos_pend.pop(p)
                rnd = rfp.tile([PE_SC, 2 * CH], F32, tag="rf")
                nc.vector.tensor_scalar(rnd[:], tps[:], MAGIC, MAGIC,
                                        AluOpType.add, AluOpType.subtract)
                frac = rfp.tile([PE_SC, 2 * CH], F32, tag="rf")
                nc.vector.tensor_tensor(frac[:], tps[:], rnd[:], AluOpType.subtract)
                sc = scp.tile([PE_SC, 2 * CH], MDT, tag="sc")
                nc.scalar.activation(sc[:], frac[:], ACT_SIN, scale=TWO_PI)
                st_sc[2 * p] = (sc, 0)
                st_sc[2 * p + 1] = (sc, 1)

            def emit_pos_elt(p):
                emit_pos_mm(p)
                emit_pos_sin(p)

            def emit_layer(l, c):
                pool = x4p if l == 4 else xp
                xdt = F16 if l == 4 else MDT
                x = pool.tile([128, 2 * CH], xdt, tag="x4" if l == 4 else "x")
                ps = midp.tile([128, 2 * CH], F32, tag="ps")
                m = modes[l]
                if m.startswith("mix"):
                    m = "split" if (c % gsz) < int(m[3:]) else "act"
                for ob in range(2):
                    osl = slice(ob * CH, (ob + 1) * CH)
                    wsl = slice(ob * 128, (ob + 1) * 128)
                    if l == 0:
                        sc, h = st_sc[c]
                        nc.tensor.matmul(ps[:, osl], w0s_sb[:, wsl],
                                         sc[:, h * CH:(h + 1) * CH],
                                         start=True, stop=True)
                    else:
                        wt = wmid_sb[l][tidx[l][c]]
                        x_in = st_x[c]
                        for kb in range(2):
                            nc.tensor.matmul(
                                ps[:, osl], wt[kb][:, wsl],
                                x_in[:, kb * CH:(kb + 1) * CH],
                                start=(kb == 0), stop=(kb == 1))
                    if m == "acts":
                        nc.scalar.activation(x[:, osl], ps[:, osl], PRELU,
                                             alpha=0.2)
                    elif m in ("split", "splitp"):
                        if ob == 0:
                            nc.scalar.activation(x[:, osl], ps[:, osl], PRELU,
                                                 alpha=0.2)
                        elif m == "split":
                            cc = cpp.tile([128, CH], F16 if l == 4 else MDT, tag="cc")
                            nc.vector.tensor_scalar(cc[:], ps[:, osl], 1.0, None,
                                                    AluOpType.mult)
                            nc.vector.scalar_tensor_tensor(
                                x[:, osl], cc[:], 0.2, cc[:],
                                AluOpType.mult, AluOpType.max)
                        else:
                            # splitp: DVE 0.2*ps copy; GPSIMD (fp32) relu+add
                            cc = cpp.tile([128, CH], F32, tag="ccp")
                            nc.vector.tensor_scalar(cc[:], ps[:, osl], 0.2, None,
                                                    AluOpType.mult)
                            tt = cpp.tile([128, CH], F32, tag="ttp")
                            nc.gpsimd.tensor_scalar(tt[:], cc[:], 0.0, 4.0,
                                                    AluOpType.max, AluOpType.mult)
                            nc.gpsimd.tensor_tensor(x[:, osl], cc[:], tt[:],
                                                    AluOpType.add)
                if m == "act":
                    nc.scalar.activation(x[:], ps[:], PRELU, alpha=0.2)
                elif m == "dvefull":
                    cc = cpp.tile([128, 2 * CH], F16 if l == 4 else MDT, tag="cc2")
                    nc.vector.tensor_scalar(cc[:], ps[:], 1.0, None,
                                            AluOpType.mult)
                    nc.vector.scalar_tensor_tensor(x[:], cc[:], 0.2, cc[:],
                                                   AluOpType.mult, AluOpType.max)
                elif m == "adve":
                    rt = cpp.tile([128, 2 * CH], F32, tag="rt")
                    nc.scalar.activation(rt[:], ps[:], RELU, scale=0.8)
                    nc.vector.scalar_tensor_tensor(x[:], ps[:], 0.2, rt[:],
                                                   AluOpType.mult, AluOpType.add)
                st_x[c] = x
                if l == 4:
                    x4[c] = x

            def emit_burst(g):
                for blk in range(gsz // 4):
                    cs = [g * gsz + blk * 4 + i for i in range(4)]
                    lps = midp.tile([128, 2 * CH], F32, tag="ps")
                    for kb in range(2):
                        for i, c in enumerate(cs):
                            nc.tensor.matmul(
                                lps[32 * i:32 * i + 3, 0:CH], wl_sb[kb][:],
                                x4[c][:, kb * CH:(kb + 1) * CH],
                                start=(kb == 0), stop=(kb == 1),
                                tile_position=(0, 32 * i))
                    # evacuate all 4 col-groups in ONE copy (partition count
                    # is free on DVE); per-chunk DMAs pick out rows 32i..32i+2
                    ot = otp.tile([99, CH], F32, tag="ot")
                    nc.vector.tensor_scalar(ot[:], lps[0:99, 0:CH], 1.0, None,
                                            AluOpType.mult)
                    for i, c in enumerate(cs):
                        bulk.dma_start(out=d_out[:, c * CH:(c + 1) * CH],
                                       in_=ot[32 * i:32 * i + 3, :])
                        del x4[c]

            npp = gsz // 2              # pairs per group
            for p in range(npp):
                emit_pos_elt(p)
            for g in range(ngr):
                for l in range(5):
                    for c in range(g * gsz, (g + 1) * gsz):
                        emit_layer(l, c)
                    if l == 0:
                        for p in range(npp // 2):
                            emit_pos_mm(npp * (g + 1) + p)
                    elif l == 1:
                        if g >= 1:
                            emit_burst(g - 1)
                        for p in range(npp // 2):
                            emit_pos_sin(npp * (g + 1) + p)
                    elif l == 2:
                        for p in range(npp // 2, npp):
                            emit_pos_mm(npp * (g + 1) + p)
                    elif l == 3:
                        for p in range(npp // 2, npp):
                            emit_pos_sin(npp * (g + 1) + p)
                        dma_group_weights(g + 2)
            emit_burst(ngr - 1)
    nc.finalize()
    return nc


def _host_prep3(coords, w0, w1, w2, w3, w4, w_last, rows, mdt="f16"):
    np_mdt = {"f16": np.float16, "bf16": np.float32, "f32r": np.float32}[mdt]

    def conv(a):
        a = np.asarray(a, np.float32)
        if mdt == "bf16":
            ai = a.view(np.uint32)
            a = ((ai + 0x8000) & 0xFFFF0000).view(np.float32)
            import ml_dtypes
            return a.astype(ml_dtypes.bfloat16)
        return a.astype(np_mdt)

    coords = np.asarray(coords, np.float32)
    smat3 = np.zeros((3, PE_SC), np.float16)
    for p in range(PE_SC - 2):
        k, f, s = p >> 2, (p >> 1) & 1, p & 1
        smat3[f, p] = np.float16(2.0 ** (k - 1))
        smat3[2, p] = np.float16(0.25 if s else 0.0)
    smat3[0, PE_SC - 2] = np.float16(COORD_S)
    smat3[1, PE_SC - 1] = np.float16(COORD_S)
    smat = np.vstack([smat3, smat3])          # [6, PE_SC]
    w0 = np.asarray(w0, np.float32)[0]
    w0s = np.empty((PE_SC, H), np.float32)
    w0s[:PE_SC - 2] = w0[2:]
    w0s[PE_SC - 2:] = w0[0:2] / np.float32(2.0 * np.pi * COORD_S)
    w0s = conv(w0s)
    wlT = conv(np.ascontiguousarray(np.asarray(w_last, np.float32).T))
    wmid_full = {1: conv(w1), 2: conv(w2), 3: conv(w3), 4: conv(w4)}
    ntile = {l: max(rows // TILE_ROWS[l], 1) for l in (1, 2, 3, 4)}
    in_maps = []
    for c in range(NCORES):
        sl = coords[c * rows:(c + 1) * rows]          # [rows, 2] fp32
        hi = sl.T.astype(np.float16)                  # [2, rows]
        lo = (sl.T - hi.astype(np.float32)).astype(np.float16)
        c6 = np.zeros((6, rows), np.float16)
        c6[0:2] = hi
        c6[2] = np.float16(1.0)
        c6[3:5] = lo
        c6r = c6.reshape(6, rows // CH, CH)
        m = {"c6e": np.ascontiguousarray(c6r[:, 0::2].reshape(6, rows // 2)),
             "c6o": np.ascontiguousarray(c6r[:, 1::2].reshape(6, rows // 2)),
             "smat": smat, "w0s": w0s, "wlT": wlT}
        for l in (1, 2, 3, 4):
            w = wmid_full[l]
            t0 = (c * rows) // (N // w.shape[0])
            m[f"w{l}"] = np.ascontiguousarray(w[t0:t0 + ntile[l]])
        in_maps.append(m)
    return in_maps


_BUILT3 = {}


def kernel(coords, w0, b0, w1, b1, w2, b2, w3, b3, w4, b4, w_last, b_last,
           modes=("act", "act", "mix2", "split", "split"), mdt="f16",
           out_eng="ADAD", psum=(0, 4), gsz=G, burst_l=1, bulk_dma="sync",
           bufbump=0):
    key = (ROWS, tuple(modes), mdt, out_eng, psum, gsz, burst_l, bulk_dma,
           bufbump)
    if key not in _BUILT3:
        _BUILT3[key] = _build3(ROWS, modes=modes, mdt=mdt, out_eng=out_eng,
                               psum=psum, gsz=gsz, burst_l=burst_l,
                               bulk_dma=bulk_dma, bufbump=bufbump)
    nc = _BUILT3[key]
    in_maps = _host_prep3(coords, w0, w1, w2, w3, w4, w_last, ROWS, mdt=mdt)
    res = run_bass_kernel_spmd(nc, in_maps, list(range(NCORES)), trace=TRACE)
    LAST["res"] = res
    out = np.empty((N, 3), np.float32)
    for c in range(NCORES):
        out[c * ROWS:(c + 1) * ROWS, :] = res.results[c]["out"].T
    return out
